# revision 22
# baseline (speedup 1.0000x reference)
"""Trainium2 Bass kernel for a dendritic layer:

    h = leaky(x @ Wd.T + bd)   # [B, 32768], Wd [32768, 1024]
    y = leaky(h @ Ws.T + bs)   # [B, 2048],  Ws [2048, 32768] block-diagonal

Sharding: tensor-parallel over the n_soma_connections axis. Core c owns
dendrites [c*4096, (c+1)*4096) == neurons [c*256, (c+1)*256), so the soma
stage is core-local (no cross-device reduction). The soma matmul collapses
to a per-column scale + segment-sum of 16 because Ws is block-diagonal.

Per core: one [256, 1024] @ [1024, 4096] GEMM on the tensor engine
(k-tiled into PSUM), dendrite bias fed into PSUM via a K=1 ones-row
matmul, then: leaky on the Scalar engine (Prelu, exact on HW), multiply by
the flattened soma weights (broadcast tile built on-device via ones-row
matmuls), segment-sum groups of 16 on the Vector engine, soma bias +
leaky, DMA out. Per-column constants are broadcast across partitions with
K=1 matmuls (ones[1,128].T @ row[1,N]) to keep HBM traffic at the
weight-stream floor.
"""

import os

import numpy as np

B = 256
IN_DIM = 1024
N_NEURONS = 2048
N_DENDRITES = 16
NSC = N_DENDRITES * N_NEURONS  # 32768
NCORES = 8
DSH = NSC // NCORES  # 4096 dendrites per core
NSH = N_NEURONS // NCORES  # 256 neurons per core
KT = IN_DIM // 128  # 8 k-tiles
NCH = DSH // 512  # 8 n-chunks of 512
BT = B // 128  # 2 batch tiles

# matmul dtype variant: "fp32" (exact, 4 cyc/row), "fp32r" (~tf32 mantissa,
# 1 cyc/row), "bf16" (host-cast weights, halves HBM)
VARIANT = os.environ.get("DK_VARIANT", "fp32r")


def _build_program(variant: str):
    import concourse.bacc as bacc
    import concourse.mybir as mybir
    import concourse.tile as tile

    f32 = mybir.dt.float32
    bf16 = mybir.dt.bfloat16
    # fp32r tensors are declared fp32r end-to-end (DRAM through SBUF) so
    # walrus sees fp32r-typed producers.
    in_dt = {"fp32": f32, "fp32r": mybir.dt.float32r, "bf16": bf16}[variant]
    add_op = mybir.AluOpType.add
    mult_op = mybir.AluOpType.mult
    max_op = mybir.AluOpType.max
    prelu = mybir.ActivationFunctionType.Prelu

    nc = bacc.Bacc("TRN2", target_bir_lowering=False, debug=False)

    xt_ap = nc.dram_tensor("xt", [128, KT, 128 * BT], in_dt, kind="ExternalInput").ap()
    # [nch, p, kt, n]: per-partition 16KB contiguous reads for full HBM eff.
    wdt_ap = nc.dram_tensor(
        "wdt", [NCH, 128, KT, 512], in_dt, kind="ExternalInput"
    ).ap()
    bd_ap = nc.dram_tensor("bd", [1, DSH], in_dt, kind="ExternalInput").ap()
    wsr_ap = nc.dram_tensor("wsr", [1, DSH], in_dt, kind="ExternalInput").ap()
    bsr_ap = nc.dram_tensor("bsr", [1, NSH], in_dt, kind="ExternalInput").ap()
    ones_ap = nc.dram_tensor("ones", [1, 128], in_dt, kind="ExternalInput").ap()
    y_ap = nc.dram_tensor("y", [B, NSH], f32, kind="ExternalOutput").ap()

    with tile.TileContext(nc) as tc:
        with (
            tc.tile_pool(name="const", bufs=1) as cpool,
            tc.tile_pool(name="xp", bufs=1) as xpool,
            tc.tile_pool(name="wp", bufs=4) as wpool,
            tc.tile_pool(name="ps", bufs=5, space="PSUM") as pspool,
            tc.tile_pool(name="psb", bufs=2, space="PSUM") as psbpool,
            tc.tile_pool(name="hp", bufs=3) as hpool,
            tc.tile_pool(name="yp", bufs=1) as ypool,
        ):
            # tiny constant rows FIRST (33KB): the broadcast-build matmuls
            # sit at the head of the in-order PE queue, so their inputs must
            # not be stuck behind megabytes of weight DMA.
            ones_t = cpool.tile([1, 128], in_dt)
            nc.sync.dma_start(ones_t[:], ones_ap[:])
            bd_t = cpool.tile([1, DSH], in_dt)
            nc.sync.dma_start(bd_t[:], bd_ap[:])
            wsr_t = cpool.tile([1, DSH], in_dt)
            nc.sync.dma_start(wsr_t[:], wsr_ap[:])
            bsr_t = cpool.tile([1, NSH], in_dt)
            nc.sync.dma_start(bsr_t[:], bsr_ap[:])

            # x (stationary operand) next — needed by every main matmul
            xt_t = xpool.tile([128, KT, 128 * BT], in_dt)
            nc.sync.dma_start(xt_t[:], xt_ap[:])

            # get the weight stream going: prefetch first chunks
            wts = []
            for nch in range(min(3, NCH)):
                wt = wpool.tile([128, KT, 512], in_dt, tag="w")
                nc.sync.dma_start(wt[:], wdt_ap[nch])
                wts.append(wt)

            # broadcast soma weights / soma bias across partitions via K=1
            # matmuls (cheap PE work that fills the warmup window while the
            # first weight chunks stream in)
            wsb_t = cpool.tile([128, DSH], f32)
            for nch in range(NCH):
                dsl = slice(nch * 512, (nch + 1) * 512)
                psw = psbpool.tile([128, 512], f32, tag="psb")
                nc.tensor.matmul(
                    psw[:], lhsT=ones_t[:], rhs=wsr_t[:, dsl], start=True, stop=True
                )
                nc.scalar.copy(wsb_t[:, dsl], psw[:])
            bsb_t = cpool.tile([128, NSH], f32)
            psb = psbpool.tile([128, NSH], f32, tag="psb")
            nc.tensor.matmul(
                psb[:], lhsT=ones_t[:], rhs=bsr_t[:], start=True, stop=True
            )
            nc.scalar.copy(bsb_t[:], psb[:])

            ypre = []
            for b in range(BT):
                yt = ypool.tile([128, NSH], f32, tag=f"ypre{b}")
                ypre.append(yt)

            for nch in range(NCH):
                if nch < len(wts):
                    wt = wts[nch]
                else:
                    wt = wpool.tile([128, KT, 512], in_dt, tag="w")
                    nc.sync.dma_start(wt[:], wdt_ap[nch])
                dsl = slice(nch * 512, (nch + 1) * 512)
                for b in range(BT):
                    ps = pspool.tile([128, 512], f32, tag="ps")
                    # dendrite bias -> PSUM via K=1 matmul (no wt/xt dep, so
                    # the scheduler can hoist it into DMA-wait windows)
                    nc.tensor.matmul(
                        ps[:], lhsT=ones_t[:], rhs=bd_t[:, dsl],
                        start=True, stop=False,
                    )
                    for kt in range(KT):
                        nc.tensor.matmul(
                            ps[:],
                            lhsT=xt_t[:, kt, b * 128 : (b + 1) * 128],
                            rhs=wt[:, kt, :],
                            start=False,
                            stop=(kt == KT - 1),
                        )
                    # leaky relu on the Scalar engine (Prelu alpha exact on HW)
                    lh = hpool.tile([128, 512], f32, tag="lh")
                    nc.scalar.activation(lh[:], ps[:], prelu, alpha=0.1)
                    # * soma weights
                    tw = hpool.tile([128, 512], f32, tag="tw")
                    nc.vector.tensor_tensor(tw[:], lh[:], wsb_t[:, dsl], mult_op)
                    # segment-sum groups of 16 -> 32 neurons per chunk
                    nc.vector.tensor_reduce(
                        ypre[b][:, nch * 32 : (nch + 1) * 32],
                        tw[:].rearrange("p (g j) -> p g j", j=16),
                        axis=mybir.AxisListType.X,
                        op=add_op,
                    )

            for b in range(BT):
                ys = hpool.tile([128, NSH], f32, tag="ys")
                nc.vector.tensor_tensor(ys[:], ypre[b][:], bsb_t[:], add_op)
                yo = hpool.tile([128, NSH], f32, tag="yo")
                nc.vector.scalar_tensor_tensor(
                    yo[:], ys[:], 0.1, ys[:], mult_op, max_op
                )
                nc.sync.dma_start(y_ap[b * 128 : (b + 1) * 128, :], yo[:])

    nc.compile()
    return nc


def _prep_inputs(x, Wd, bd, Ws, bs, variant: str):
    """Build the per-core input maps (host-side shard + relayout)."""
    in_np = np.dtype(np.float32)
    if variant == "bf16":
        import ml_dtypes

        in_np = np.dtype(ml_dtypes.bfloat16)

    # xt[p, kt, m] = x[m, kt*128+p]
    xt = np.ascontiguousarray(x.T.reshape(KT, 128, B).transpose(1, 0, 2)).astype(in_np)
    ones = np.ones((1, 128), dtype=in_np)

    in_maps = []
    for c in range(NCORES):
        dsl = slice(c * DSH, (c + 1) * DSH)
        nsl = slice(c * NSH, (c + 1) * NSH)
        # wdt[nch, p, kt, n] = Wd[c*DSH + nch*512 + n, kt*128 + p]
        wdt = np.ascontiguousarray(
            Wd[dsl].T.reshape(KT, 128, NCH, 512).transpose(2, 1, 0, 3)
        ).astype(in_np)
        blk = Ws[nsl, dsl]  # [256, 4096] diagonal blocks
        ws_flat = blk.reshape(NSH, NSH, N_DENDRITES)[
            np.arange(NSH), np.arange(NSH), :
        ].reshape(1, DSH)
        in_maps.append(
            {
                "xt": xt,
                "wdt": wdt,
                "ones": ones,
                "bd": np.ascontiguousarray(bd[dsl]).reshape(1, DSH).astype(in_np),
                "wsr": np.ascontiguousarray(ws_flat).astype(in_np),
                "bsr": np.ascontiguousarray(bs[nsl]).reshape(1, NSH).astype(in_np),
            }
        )
    return in_maps


_cache = {}


def run(x, Wd, bd, Ws, bs, variant=None, trace=False):
    from concourse.bass_utils import run_bass_kernel_spmd

    variant = variant or VARIANT
    if variant not in _cache:
        _cache[variant] = _build_program(variant)
    nc = _cache[variant]
    in_maps = _prep_inputs(x, Wd, bd, Ws, bs, variant)
    res = run_bass_kernel_spmd(nc, in_maps, list(range(NCORES)), trace=trace)
    y = np.concatenate([res.results[c]["y"] for c in range(NCORES)], axis=1)
    return y.astype(np.float32), res


def kernel(x, Wd, bd, Ws, bs):
    y, _ = run(x, Wd, bd, Ws, bs)
    return y


# revision 23
# speedup vs baseline: 1.0286x; 1.0286x over previous
"""Trainium2 Bass kernel for a dendritic layer:

    h = leaky(x @ Wd.T + bd)   # [B, 32768], Wd [32768, 1024]
    y = leaky(h @ Ws.T + bs)   # [B, 2048],  Ws [2048, 32768] block-diagonal

Sharding: tensor-parallel over the n_soma_connections axis. Core c owns
dendrites [c*4096, (c+1)*4096) == neurons [c*256, (c+1)*256), so the soma
stage is core-local (no cross-device reduction). The soma matmul collapses
to a per-column scale + segment-sum of 16 because Ws is block-diagonal.

Per core: one [256, 1024] @ [1024, 4096] GEMM on the tensor engine
(k-tiled into PSUM), dendrite bias fed into PSUM via a K=1 ones-row
matmul, then: leaky on the Scalar engine (Prelu, exact on HW), multiply by
the flattened soma weights (broadcast tile built on-device via ones-row
matmuls), segment-sum groups of 16 on the Vector engine, soma bias +
leaky, DMA out. Per-column constants are broadcast across partitions with
K=1 matmuls (ones[1,128].T @ row[1,N]) to keep HBM traffic at the
weight-stream floor.
"""

import os

import numpy as np

B = 256
IN_DIM = 1024
N_NEURONS = 2048
N_DENDRITES = 16
NSC = N_DENDRITES * N_NEURONS  # 32768
NCORES = 8
DSH = NSC // NCORES  # 4096 dendrites per core
NSH = N_NEURONS // NCORES  # 256 neurons per core
KT = IN_DIM // 128  # 8 k-tiles
NCH = DSH // 512  # 8 n-chunks of 512
BT = B // 128  # 2 batch tiles

# matmul dtype variant: "fp32" (exact, 4 cyc/row), "fp32r" (~tf32 mantissa,
# 1 cyc/row), "bf16" (host-cast weights, halves HBM)
VARIANT = os.environ.get("DK_VARIANT", "fp32r")


def _build_program(variant: str):
    import concourse.bacc as bacc
    import concourse.mybir as mybir
    import concourse.tile as tile

    f32 = mybir.dt.float32
    bf16 = mybir.dt.bfloat16
    # fp32r tensors are declared fp32r end-to-end (DRAM through SBUF) so
    # walrus sees fp32r-typed producers.
    in_dt = {"fp32": f32, "fp32r": mybir.dt.float32r, "bf16": bf16}[variant]
    add_op = mybir.AluOpType.add
    mult_op = mybir.AluOpType.mult
    max_op = mybir.AluOpType.max
    prelu = mybir.ActivationFunctionType.Prelu

    nc = bacc.Bacc("TRN2", target_bir_lowering=False, debug=False)

    xt_ap = nc.dram_tensor("xt", [128, KT, 128 * BT], in_dt, kind="ExternalInput").ap()
    # [nch, p, kt, n]: per-partition 16KB contiguous reads for full HBM eff.
    wdt_ap = nc.dram_tensor(
        "wdt", [NCH, 128, KT, 512], in_dt, kind="ExternalInput"
    ).ap()
    bd_ap = nc.dram_tensor("bd", [1, DSH], in_dt, kind="ExternalInput").ap()
    wsr_ap = nc.dram_tensor("wsr", [1, DSH], in_dt, kind="ExternalInput").ap()
    bsr_ap = nc.dram_tensor("bsr", [1, NSH], in_dt, kind="ExternalInput").ap()
    ones_ap = nc.dram_tensor("ones", [1, 128], in_dt, kind="ExternalInput").ap()
    y_ap = nc.dram_tensor("y", [B, NSH], f32, kind="ExternalOutput").ap()

    with tile.TileContext(nc) as tc:
        with (
            tc.tile_pool(name="const", bufs=1) as cpool,
            tc.tile_pool(name="xp", bufs=1) as xpool,
            tc.tile_pool(name="wp", bufs=6) as wpool,
            tc.tile_pool(name="ps", bufs=5, space="PSUM") as pspool,
            tc.tile_pool(name="psb", bufs=2, space="PSUM") as psbpool,
            tc.tile_pool(name="hp", bufs=3) as hpool,
            tc.tile_pool(name="yp", bufs=1) as ypool,
        ):
            # tiny constant rows FIRST (33KB): the broadcast-build matmuls
            # sit at the head of the in-order PE queue, so their inputs must
            # not be stuck behind megabytes of weight DMA.
            ones_t = cpool.tile([1, 128], in_dt)
            nc.sync.dma_start(ones_t[:], ones_ap[:])
            bd_t = cpool.tile([1, DSH], in_dt)
            nc.sync.dma_start(bd_t[:], bd_ap[:])
            wsr_t = cpool.tile([1, DSH], in_dt)
            nc.sync.dma_start(wsr_t[:], wsr_ap[:])
            bsr_t = cpool.tile([1, NSH], in_dt)
            nc.sync.dma_start(bsr_t[:], bsr_ap[:])

            # x (stationary operand) next — needed by every main matmul
            xt_t = xpool.tile([128, KT, 128 * BT], in_dt)
            nc.sync.dma_start(xt_t[:], xt_ap[:])

            # get the weight stream going: prefetch first chunks
            wts = []
            for nch in range(min(5, NCH)):
                wt = wpool.tile([128, KT, 512], in_dt, tag="w")
                nc.sync.dma_start(wt[:], wdt_ap[nch])
                wts.append(wt)

            # broadcast soma weights / soma bias across partitions via K=1
            # matmuls (cheap PE work that fills the warmup window while the
            # first weight chunks stream in)
            wsb_t = cpool.tile([128, DSH], f32)
            for nch in range(NCH):
                dsl = slice(nch * 512, (nch + 1) * 512)
                psw = psbpool.tile([128, 512], f32, tag="psb")
                nc.tensor.matmul(
                    psw[:], lhsT=ones_t[:], rhs=wsr_t[:, dsl], start=True, stop=True
                )
                nc.scalar.copy(wsb_t[:, dsl], psw[:])
            bsb_t = cpool.tile([128, NSH], f32)
            psb = psbpool.tile([128, NSH], f32, tag="psb")
            nc.tensor.matmul(
                psb[:], lhsT=ones_t[:], rhs=bsr_t[:], start=True, stop=True
            )
            nc.scalar.copy(bsb_t[:], psb[:])

            ypre = []
            for b in range(BT):
                yt = ypool.tile([128, NSH], f32, tag=f"ypre{b}")
                ypre.append(yt)

            for nch in range(NCH):
                if nch < len(wts):
                    wt = wts[nch]
                else:
                    wt = wpool.tile([128, KT, 512], in_dt, tag="w")
                    nc.sync.dma_start(wt[:], wdt_ap[nch])
                dsl = slice(nch * 512, (nch + 1) * 512)
                for b in range(BT):
                    ps = pspool.tile([128, 512], f32, tag="ps")
                    # dendrite bias -> PSUM via K=1 matmul (no wt/xt dep, so
                    # the scheduler can hoist it into DMA-wait windows)
                    nc.tensor.matmul(
                        ps[:], lhsT=ones_t[:], rhs=bd_t[:, dsl],
                        start=True, stop=False,
                    )
                    for kt in range(KT):
                        nc.tensor.matmul(
                            ps[:],
                            lhsT=xt_t[:, kt, b * 128 : (b + 1) * 128],
                            rhs=wt[:, kt, :],
                            start=False,
                            stop=(kt == KT - 1),
                        )
                    # leaky relu on the Scalar engine (Prelu alpha exact on HW)
                    lh = hpool.tile([128, 512], f32, tag="lh")
                    nc.scalar.activation(lh[:], ps[:], prelu, alpha=0.1)
                    # * soma weights
                    tw = hpool.tile([128, 512], f32, tag="tw")
                    nc.vector.tensor_tensor(tw[:], lh[:], wsb_t[:, dsl], mult_op)
                    # segment-sum groups of 16 -> 32 neurons per chunk
                    nc.vector.tensor_reduce(
                        ypre[b][:, nch * 32 : (nch + 1) * 32],
                        tw[:].rearrange("p (g j) -> p g j", j=16),
                        axis=mybir.AxisListType.X,
                        op=add_op,
                    )

            for b in range(BT):
                ys = hpool.tile([128, NSH], f32, tag="ys")
                nc.vector.tensor_tensor(ys[:], ypre[b][:], bsb_t[:], add_op)
                yo = hpool.tile([128, NSH], f32, tag="yo")
                nc.vector.scalar_tensor_tensor(
                    yo[:], ys[:], 0.1, ys[:], mult_op, max_op
                )
                nc.sync.dma_start(y_ap[b * 128 : (b + 1) * 128, :], yo[:])

    nc.compile()
    return nc


def _prep_inputs(x, Wd, bd, Ws, bs, variant: str):
    """Build the per-core input maps (host-side shard + relayout)."""
    in_np = np.dtype(np.float32)
    if variant == "bf16":
        import ml_dtypes

        in_np = np.dtype(ml_dtypes.bfloat16)

    # xt[p, kt, m] = x[m, kt*128+p]
    xt = np.ascontiguousarray(x.T.reshape(KT, 128, B).transpose(1, 0, 2)).astype(in_np)
    ones = np.ones((1, 128), dtype=in_np)

    in_maps = []
    for c in range(NCORES):
        dsl = slice(c * DSH, (c + 1) * DSH)
        nsl = slice(c * NSH, (c + 1) * NSH)
        # wdt[nch, p, kt, n] = Wd[c*DSH + nch*512 + n, kt*128 + p]
        wdt = np.ascontiguousarray(
            Wd[dsl].T.reshape(KT, 128, NCH, 512).transpose(2, 1, 0, 3)
        ).astype(in_np)
        blk = Ws[nsl, dsl]  # [256, 4096] diagonal blocks
        ws_flat = blk.reshape(NSH, NSH, N_DENDRITES)[
            np.arange(NSH), np.arange(NSH), :
        ].reshape(1, DSH)
        in_maps.append(
            {
                "xt": xt,
                "wdt": wdt,
                "ones": ones,
                "bd": np.ascontiguousarray(bd[dsl]).reshape(1, DSH).astype(in_np),
                "wsr": np.ascontiguousarray(ws_flat).astype(in_np),
                "bsr": np.ascontiguousarray(bs[nsl]).reshape(1, NSH).astype(in_np),
            }
        )
    return in_maps


_cache = {}


def run(x, Wd, bd, Ws, bs, variant=None, trace=False):
    from concourse.bass_utils import run_bass_kernel_spmd

    variant = variant or VARIANT
    if variant not in _cache:
        _cache[variant] = _build_program(variant)
    nc = _cache[variant]
    in_maps = _prep_inputs(x, Wd, bd, Ws, bs, variant)
    res = run_bass_kernel_spmd(nc, in_maps, list(range(NCORES)), trace=trace)
    y = np.concatenate([res.results[c]["y"] for c in range(NCORES)], axis=1)
    return y.astype(np.float32), res


def kernel(x, Wd, bd, Ws, bs):
    y, _ = run(x, Wd, bd, Ws, bs)
    return y


# revision 25
# speedup vs baseline: 1.2434x; 1.2088x over previous
"""Trainium2 Bass kernel for a dendritic layer:

    h = leaky(x @ Wd.T + bd)   # [B, 32768], Wd [32768, 1024]
    y = leaky(h @ Ws.T + bs)   # [B, 2048],  Ws [2048, 32768] block-diagonal

Sharding: tensor-parallel over the n_soma_connections axis. Core c owns
dendrites [c*4096, (c+1)*4096) == neurons [c*256, (c+1)*256), so the soma
stage is core-local (no cross-device reduction). The soma matmul collapses
to a per-column scale + segment-sum of 16 because Ws is block-diagonal.

Per core: one [256, 1024] @ [1024, 4096] GEMM on the tensor engine
(k-tiled into PSUM), dendrite bias fed into PSUM via a K=1 ones-row
matmul, then: leaky on the Scalar engine (Prelu, exact on HW), multiply by
the flattened soma weights (broadcast tile built on-device via ones-row
matmuls), segment-sum groups of 16 on the Vector engine, soma bias +
leaky, DMA out. Per-column constants are broadcast across partitions with
K=1 matmuls (ones[1,128].T @ row[1,N]) to keep HBM traffic at the
weight-stream floor.
"""

import os

import numpy as np

B = 256
IN_DIM = 1024
N_NEURONS = 2048
N_DENDRITES = 16
NSC = N_DENDRITES * N_NEURONS  # 32768
NCORES = 8
DSH = NSC // NCORES  # 4096 dendrites per core
NSH = N_NEURONS // NCORES  # 256 neurons per core
KT = IN_DIM // 128  # 8 k-tiles
NCH = DSH // 512  # 8 n-chunks of 512
BT = B // 128  # 2 batch tiles

# matmul dtype variant: "fp32" (exact, 4 cyc/row), "fp32r" (~tf32 mantissa,
# 1 cyc/row), "bf16" (host-cast weights, halves HBM)
VARIANT = os.environ.get("DK_VARIANT", "fp32r")


def _build_program(variant: str):
    import concourse.bacc as bacc
    import concourse.mybir as mybir
    import concourse.tile as tile

    f32 = mybir.dt.float32
    bf16 = mybir.dt.bfloat16
    # fp32r tensors are declared fp32r end-to-end (DRAM through SBUF) so
    # walrus sees fp32r-typed producers.
    in_dt = {
        "fp32": f32,
        "fp32r": mybir.dt.float32r,
        "bf16": bf16,
        "fp16": mybir.dt.float16,
    }[variant]
    add_op = mybir.AluOpType.add
    mult_op = mybir.AluOpType.mult
    max_op = mybir.AluOpType.max
    prelu = mybir.ActivationFunctionType.Prelu

    nc = bacc.Bacc("TRN2", target_bir_lowering=False, debug=False)

    xt_ap = nc.dram_tensor("xt", [128, KT, 128 * BT], in_dt, kind="ExternalInput").ap()
    # [nch, p, kt, n]: per-partition 16KB contiguous reads for full HBM eff.
    wdt_ap = nc.dram_tensor(
        "wdt", [NCH, 128, KT, 512], in_dt, kind="ExternalInput"
    ).ap()
    bd_ap = nc.dram_tensor("bd", [1, DSH], in_dt, kind="ExternalInput").ap()
    wsr_ap = nc.dram_tensor("wsr", [1, DSH], in_dt, kind="ExternalInput").ap()
    bsr_ap = nc.dram_tensor("bsr", [1, NSH], in_dt, kind="ExternalInput").ap()
    ones_ap = nc.dram_tensor("ones", [1, 128], in_dt, kind="ExternalInput").ap()
    y_ap = nc.dram_tensor("y", [B, NSH], f32, kind="ExternalOutput").ap()

    with tile.TileContext(nc) as tc:
        with (
            tc.tile_pool(name="const", bufs=1) as cpool,
            tc.tile_pool(name="xp", bufs=1) as xpool,
            tc.tile_pool(name="wp", bufs=6) as wpool,
            tc.tile_pool(name="ps", bufs=5, space="PSUM") as pspool,
            tc.tile_pool(name="psb", bufs=2, space="PSUM") as psbpool,
            tc.tile_pool(name="hp", bufs=3) as hpool,
            tc.tile_pool(name="yp", bufs=1) as ypool,
        ):
            # tiny constant rows FIRST (33KB): the broadcast-build matmuls
            # sit at the head of the in-order PE queue, so their inputs must
            # not be stuck behind megabytes of weight DMA.
            ones_t = cpool.tile([1, 128], in_dt)
            nc.sync.dma_start(ones_t[:], ones_ap[:])
            bd_t = cpool.tile([1, DSH], in_dt)
            nc.sync.dma_start(bd_t[:], bd_ap[:])
            wsr_t = cpool.tile([1, DSH], in_dt)
            nc.sync.dma_start(wsr_t[:], wsr_ap[:])
            bsr_t = cpool.tile([1, NSH], in_dt)
            nc.sync.dma_start(bsr_t[:], bsr_ap[:])

            # x (stationary operand) next — needed by every main matmul
            xt_t = xpool.tile([128, KT, 128 * BT], in_dt)
            nc.sync.dma_start(xt_t[:], xt_ap[:])

            # get the weight stream going: prefetch first chunks
            wts = []
            for nch in range(min(5, NCH)):
                wt = wpool.tile([128, KT, 512], in_dt, tag="w")
                nc.sync.dma_start(wt[:], wdt_ap[nch])
                wts.append(wt)

            # broadcast soma weights / soma bias across partitions via K=1
            # matmuls (cheap PE work that fills the warmup window while the
            # first weight chunks stream in)
            wsb_t = cpool.tile([128, DSH], f32)
            for nch in range(NCH):
                dsl = slice(nch * 512, (nch + 1) * 512)
                psw = psbpool.tile([128, 512], f32, tag="psb")
                nc.tensor.matmul(
                    psw[:], lhsT=ones_t[:], rhs=wsr_t[:, dsl], start=True, stop=True
                )
                nc.scalar.copy(wsb_t[:, dsl], psw[:])
            bsb_t = cpool.tile([128, NSH], f32)
            psb = psbpool.tile([128, NSH], f32, tag="psb")
            nc.tensor.matmul(
                psb[:], lhsT=ones_t[:], rhs=bsr_t[:], start=True, stop=True
            )
            nc.scalar.copy(bsb_t[:], psb[:])

            ypre = []
            for b in range(BT):
                yt = ypool.tile([128, NSH], f32, tag=f"ypre{b}")
                ypre.append(yt)

            for nch in range(NCH):
                if nch < len(wts):
                    wt = wts[nch]
                else:
                    wt = wpool.tile([128, KT, 512], in_dt, tag="w")
                    nc.sync.dma_start(wt[:], wdt_ap[nch])
                dsl = slice(nch * 512, (nch + 1) * 512)
                for b in range(BT):
                    ps = pspool.tile([128, 512], f32, tag="ps")
                    # dendrite bias -> PSUM via K=1 matmul (no wt/xt dep, so
                    # the scheduler can hoist it into DMA-wait windows)
                    nc.tensor.matmul(
                        ps[:], lhsT=ones_t[:], rhs=bd_t[:, dsl],
                        start=True, stop=False,
                    )
                    for kt in range(KT):
                        nc.tensor.matmul(
                            ps[:],
                            lhsT=xt_t[:, kt, b * 128 : (b + 1) * 128],
                            rhs=wt[:, kt, :],
                            start=False,
                            stop=(kt == KT - 1),
                        )
                    # leaky relu on the Scalar engine (Prelu alpha exact on HW)
                    lh = hpool.tile([128, 512], f32, tag="lh")
                    nc.scalar.activation(lh[:], ps[:], prelu, alpha=0.1)
                    # * soma weights
                    tw = hpool.tile([128, 512], f32, tag="tw")
                    nc.vector.tensor_tensor(tw[:], lh[:], wsb_t[:, dsl], mult_op)
                    # segment-sum groups of 16 -> 32 neurons per chunk
                    nc.vector.tensor_reduce(
                        ypre[b][:, nch * 32 : (nch + 1) * 32],
                        tw[:].rearrange("p (g j) -> p g j", j=16),
                        axis=mybir.AxisListType.X,
                        op=add_op,
                    )

            for b in range(BT):
                ys = hpool.tile([128, NSH], f32, tag="ys")
                nc.vector.tensor_tensor(ys[:], ypre[b][:], bsb_t[:], add_op)
                yo = hpool.tile([128, NSH], f32, tag="yo")
                nc.vector.scalar_tensor_tensor(
                    yo[:], ys[:], 0.1, ys[:], mult_op, max_op
                )
                nc.sync.dma_start(y_ap[b * 128 : (b + 1) * 128, :], yo[:])

    nc.compile()
    return nc


def _prep_inputs(x, Wd, bd, Ws, bs, variant: str):
    """Build the per-core input maps (host-side shard + relayout)."""
    in_np = np.dtype(np.float32)
    if variant == "bf16":
        import ml_dtypes

        in_np = np.dtype(ml_dtypes.bfloat16)
    elif variant == "fp16":
        in_np = np.dtype(np.float16)

    # xt[p, kt, m] = x[m, kt*128+p]
    xt = np.ascontiguousarray(x.T.reshape(KT, 128, B).transpose(1, 0, 2)).astype(in_np)
    ones = np.ones((1, 128), dtype=in_np)

    in_maps = []
    for c in range(NCORES):
        dsl = slice(c * DSH, (c + 1) * DSH)
        nsl = slice(c * NSH, (c + 1) * NSH)
        # wdt[nch, p, kt, n] = Wd[c*DSH + nch*512 + n, kt*128 + p]
        wdt = np.ascontiguousarray(
            Wd[dsl].T.reshape(KT, 128, NCH, 512).transpose(2, 1, 0, 3)
        ).astype(in_np)
        blk = Ws[nsl, dsl]  # [256, 4096] diagonal blocks
        ws_flat = blk.reshape(NSH, NSH, N_DENDRITES)[
            np.arange(NSH), np.arange(NSH), :
        ].reshape(1, DSH)
        in_maps.append(
            {
                "xt": xt,
                "wdt": wdt,
                "ones": ones,
                "bd": np.ascontiguousarray(bd[dsl]).reshape(1, DSH).astype(in_np),
                "wsr": np.ascontiguousarray(ws_flat).astype(in_np),
                "bsr": np.ascontiguousarray(bs[nsl]).reshape(1, NSH).astype(in_np),
            }
        )
    return in_maps


_cache = {}


def run(x, Wd, bd, Ws, bs, variant=None, trace=False):
    from concourse.bass_utils import run_bass_kernel_spmd

    variant = variant or VARIANT
    if variant not in _cache:
        _cache[variant] = _build_program(variant)
    nc = _cache[variant]
    in_maps = _prep_inputs(x, Wd, bd, Ws, bs, variant)
    res = run_bass_kernel_spmd(nc, in_maps, list(range(NCORES)), trace=trace)
    y = np.concatenate([res.results[c]["y"] for c in range(NCORES)], axis=1)
    return y.astype(np.float32), res


def kernel(x, Wd, bd, Ws, bs):
    y, _ = run(x, Wd, bd, Ws, bs)
    return y


# revision 28
# speedup vs baseline: 1.2660x; 1.0181x over previous
"""Trainium2 Bass kernel for a dendritic layer:

    h = leaky(x @ Wd.T + bd)   # [B, 32768], Wd [32768, 1024]
    y = leaky(h @ Ws.T + bs)   # [B, 2048],  Ws [2048, 32768] block-diagonal

Sharding: tensor-parallel over the n_soma_connections axis. Core c owns
dendrites [c*4096, (c+1)*4096) == neurons [c*256, (c+1)*256), so the soma
stage is core-local (no cross-device reduction). The soma matmul collapses
to a per-column scale + segment-sum of 16 because Ws is block-diagonal.

Per core: one [256, 1024] @ [1024, 4096] GEMM on the tensor engine
(k-tiled into PSUM), dendrite bias fed into PSUM via a K=1 ones-row
matmul, then: leaky on the Scalar engine (Prelu, exact on HW), multiply by
the flattened soma weights (broadcast tile built on-device via ones-row
matmuls), segment-sum groups of 16 on the Vector engine, soma bias +
leaky, DMA out. Per-column constants are broadcast across partitions with
K=1 matmuls (ones[1,128].T @ row[1,N]) to keep HBM traffic at the
weight-stream floor.
"""

import os

import numpy as np

B = 256
IN_DIM = 1024
N_NEURONS = 2048
N_DENDRITES = 16
NSC = N_DENDRITES * N_NEURONS  # 32768
NCORES = 8
DSH = NSC // NCORES  # 4096 dendrites per core
NSH = N_NEURONS // NCORES  # 256 neurons per core
KT = IN_DIM // 128  # 8 k-tiles
NCH = DSH // 512  # 8 n-chunks of 512
BT = B // 128  # 2 batch tiles

# matmul dtype variant: "fp32" (exact, 4 cyc/row), "fp32r" (~tf32 mantissa,
# 1 cyc/row), "bf16" (host-cast weights, halves HBM)
VARIANT = os.environ.get("DK_VARIANT", "fp32r")


def _build_program(variant: str):
    import concourse.bacc as bacc
    import concourse.mybir as mybir
    import concourse.tile as tile

    f32 = mybir.dt.float32
    bf16 = mybir.dt.bfloat16
    # fp32r tensors are declared fp32r end-to-end (DRAM through SBUF) so
    # walrus sees fp32r-typed producers.
    in_dt = {
        "fp32": f32,
        "fp32r": mybir.dt.float32r,
        "bf16": bf16,
        "fp16": mybir.dt.float16,
    }[variant]
    add_op = mybir.AluOpType.add
    mult_op = mybir.AluOpType.mult
    max_op = mybir.AluOpType.max
    prelu = mybir.ActivationFunctionType.Prelu

    nc = bacc.Bacc("TRN2", target_bir_lowering=False, debug=False)

    xt_ap = nc.dram_tensor("xt", [128, KT, 128 * BT], in_dt, kind="ExternalInput").ap()
    # [nch, p, kt, n]: per-partition 16KB contiguous reads for full HBM eff.
    wdt_ap = nc.dram_tensor(
        "wdt", [NCH, 128, KT, 512], in_dt, kind="ExternalInput"
    ).ap()
    bd_ap = nc.dram_tensor("bd", [1, DSH], in_dt, kind="ExternalInput").ap()
    wsr_ap = nc.dram_tensor("wsr", [1, DSH], in_dt, kind="ExternalInput").ap()
    bsr_ap = nc.dram_tensor("bsr", [1, NSH], in_dt, kind="ExternalInput").ap()
    ones_ap = nc.dram_tensor("ones", [1, 128], in_dt, kind="ExternalInput").ap()
    y_ap = nc.dram_tensor("y", [B, NSH], f32, kind="ExternalOutput").ap()

    with tile.TileContext(nc) as tc:
        with (
            tc.tile_pool(name="const", bufs=1) as cpool,
            tc.tile_pool(name="xp", bufs=1) as xpool,
            tc.tile_pool(name="wp", bufs=8) as wpool,
            tc.tile_pool(name="ps", bufs=5, space="PSUM") as pspool,
            tc.tile_pool(name="psb", bufs=2, space="PSUM") as psbpool,
            tc.tile_pool(name="hp", bufs=3) as hpool,
            tc.tile_pool(name="yp", bufs=1) as ypool,
        ):
            # tiny constant rows FIRST (33KB): the broadcast-build matmuls
            # sit at the head of the in-order PE queue, so their inputs must
            # not be stuck behind megabytes of weight DMA.
            ones_t = cpool.tile([1, 128], in_dt)
            nc.sync.dma_start(ones_t[:], ones_ap[:])
            bd_t = cpool.tile([1, DSH], in_dt)
            nc.sync.dma_start(bd_t[:], bd_ap[:])
            wsr_t = cpool.tile([1, DSH], in_dt)
            nc.sync.dma_start(wsr_t[:], wsr_ap[:])
            bsr_t = cpool.tile([1, NSH], in_dt)
            nc.sync.dma_start(bsr_t[:], bsr_ap[:])

            # x (stationary operand) next — needed by every main matmul
            xt_t = xpool.tile([128, KT, 128 * BT], in_dt)
            nc.sync.dma_start(xt_t[:], xt_ap[:])

            # prefetch ALL weight chunks upfront — they fit in SBUF, and the
            # DMA stream then runs flat-out while the PE consumes in order
            wts = []
            for nch in range(NCH):
                wt = wpool.tile([128, KT, 512], in_dt, tag="w")
                nc.sync.dma_start(wt[:], wdt_ap[nch])
                wts.append(wt)

            # broadcast soma weights / soma bias across partitions via K=1
            # matmuls (cheap PE work that fills the warmup window while the
            # first weight chunks stream in)
            wsb_t = cpool.tile([128, DSH], f32)
            for nch in range(NCH):
                dsl = slice(nch * 512, (nch + 1) * 512)
                psw = psbpool.tile([128, 512], f32, tag="psb")
                nc.tensor.matmul(
                    psw[:], lhsT=ones_t[:], rhs=wsr_t[:, dsl], start=True, stop=True
                )
                nc.scalar.copy(wsb_t[:, dsl], psw[:])
            bsb_t = cpool.tile([128, NSH], f32)
            psb = psbpool.tile([128, NSH], f32, tag="psb")
            nc.tensor.matmul(
                psb[:], lhsT=ones_t[:], rhs=bsr_t[:], start=True, stop=True
            )
            nc.scalar.copy(bsb_t[:], psb[:])

            ypre = []
            for b in range(BT):
                yt = ypool.tile([128, NSH], f32, tag=f"ypre{b}")
                ypre.append(yt)

            for nch in range(NCH):
                wt = wts[nch]
                dsl = slice(nch * 512, (nch + 1) * 512)
                for b in range(BT):
                    ps = pspool.tile([128, 512], f32, tag="ps")
                    # dendrite bias -> PSUM via K=1 matmul (no wt/xt dep, so
                    # the scheduler can hoist it into DMA-wait windows)
                    nc.tensor.matmul(
                        ps[:], lhsT=ones_t[:], rhs=bd_t[:, dsl],
                        start=True, stop=False,
                    )
                    for kt in range(KT):
                        nc.tensor.matmul(
                            ps[:],
                            lhsT=xt_t[:, kt, b * 128 : (b + 1) * 128],
                            rhs=wt[:, kt, :],
                            start=False,
                            stop=(kt == KT - 1),
                        )
                    # leaky relu on the Scalar engine (Prelu alpha exact on HW)
                    lh = hpool.tile([128, 512], f32, tag="lh")
                    nc.scalar.activation(lh[:], ps[:], prelu, alpha=0.1)
                    # * soma weights
                    tw = hpool.tile([128, 512], f32, tag="tw")
                    nc.vector.tensor_tensor(tw[:], lh[:], wsb_t[:, dsl], mult_op)
                    # segment-sum groups of 16 -> 32 neurons per chunk
                    nc.vector.tensor_reduce(
                        ypre[b][:, nch * 32 : (nch + 1) * 32],
                        tw[:].rearrange("p (g j) -> p g j", j=16),
                        axis=mybir.AxisListType.X,
                        op=add_op,
                    )

            for b in range(BT):
                ys = hpool.tile([128, NSH], f32, tag="ys")
                nc.vector.tensor_tensor(ys[:], ypre[b][:], bsb_t[:], add_op)
                yo = hpool.tile([128, NSH], f32, tag="yo")
                nc.vector.scalar_tensor_tensor(
                    yo[:], ys[:], 0.1, ys[:], mult_op, max_op
                )
                nc.sync.dma_start(y_ap[b * 128 : (b + 1) * 128, :], yo[:])

    nc.compile()
    return nc


def _prep_inputs(x, Wd, bd, Ws, bs, variant: str):
    """Build the per-core input maps (host-side shard + relayout)."""
    in_np = np.dtype(np.float32)
    if variant == "bf16":
        import ml_dtypes

        in_np = np.dtype(ml_dtypes.bfloat16)
    elif variant == "fp16":
        in_np = np.dtype(np.float16)

    # xt[p, kt, m] = x[m, kt*128+p]
    xt = np.ascontiguousarray(x.T.reshape(KT, 128, B).transpose(1, 0, 2)).astype(in_np)
    ones = np.ones((1, 128), dtype=in_np)

    in_maps = []
    for c in range(NCORES):
        dsl = slice(c * DSH, (c + 1) * DSH)
        nsl = slice(c * NSH, (c + 1) * NSH)
        # wdt[nch, p, kt, n] = Wd[c*DSH + nch*512 + n, kt*128 + p]
        wdt = np.ascontiguousarray(
            Wd[dsl].T.reshape(KT, 128, NCH, 512).transpose(2, 1, 0, 3)
        ).astype(in_np)
        blk = Ws[nsl, dsl]  # [256, 4096] diagonal blocks
        ws_flat = blk.reshape(NSH, NSH, N_DENDRITES)[
            np.arange(NSH), np.arange(NSH), :
        ].reshape(1, DSH)
        in_maps.append(
            {
                "xt": xt,
                "wdt": wdt,
                "ones": ones,
                "bd": np.ascontiguousarray(bd[dsl]).reshape(1, DSH).astype(in_np),
                "wsr": np.ascontiguousarray(ws_flat).astype(in_np),
                "bsr": np.ascontiguousarray(bs[nsl]).reshape(1, NSH).astype(in_np),
            }
        )
    return in_maps


_cache = {}


def run(x, Wd, bd, Ws, bs, variant=None, trace=False):
    from concourse.bass_utils import run_bass_kernel_spmd

    variant = variant or VARIANT
    if variant not in _cache:
        _cache[variant] = _build_program(variant)
    nc = _cache[variant]
    in_maps = _prep_inputs(x, Wd, bd, Ws, bs, variant)
    res = run_bass_kernel_spmd(nc, in_maps, list(range(NCORES)), trace=trace)
    y = np.concatenate([res.results[c]["y"] for c in range(NCORES)], axis=1)
    return y.astype(np.float32), res


def kernel(x, Wd, bd, Ws, bs):
    y, _ = run(x, Wd, bd, Ws, bs)
    return y
